# revision 28
# baseline (speedup 1.0000x reference)
"""Single-head causal attention (B=4, T=4096, n_embd=1024, head=64) on 8 trn2 cores.

One SPMD program, 8 cores, one launch.  Core c -> batch b=c//2, half h=c%2.
Causal-balanced q-block (512 rows) assignment: half0 {0,3,4,7}, half1 {1,2,5,6}.

Uniform instruction stream across cores; everything core-specific is DATA:
  - xq: the core's own 4 q-blocks of x^T, host-gathered in slot order
  - masks: 16 precomputed [128,1024] 0/1 bf16 tiles (slot si, masked pair m)
All device inputs are host-pre-swizzled into the exact SBUF layout (partition-
major, contiguous free dim) so every input DMA is a plain 2D copy with 8KB
contiguous runs per partition (the naive 3D gather moved only ~1KB per packet
and crawled at ~90 GB/s).

Slot si covers SLOT_NK[si] = {8,16,24,32} k-blocks (128 keys each); the last
4 pairs of each slot are mask-multiplied (covers both the causal diagonal and
the padding when the hosted q-block needs fewer k-blocks than the slot).

Math (S^T formulation, bf16 inputs / fp32 PSUM):
  S^T[tk,tq] = K_blk^T.T @ Q^T   (the 2 k-blocks of a pair run as row-tiled
                                  64x128 matmuls on array rows 0:63 / 64:127
                                  concurrently -> 2 psum banks)
  P^T = exp(S^T / 8) -> bf16     (one ACT op over both banks)
  P^T *= mask                    (DVE tensor_tensor, bf16 2x mode)
  O_aug^T[65,512] += V_aug_blk.T @ P^T   (V_aug col 64 = ones => row 64 of
                                  O_aug accumulates the softmax denominator)

Schedule: all Q projected up front (col-tiled M=64 pairs replicate Q^T to both
partition halves for the row-tiled S); attention pairs are emitted
incrementally one t-block after their keys are projected (slot0@tb1-2,
slot1@tb3-4, slot2@tb5-6, slot3 2 pairs/tb) and SOFTWARE-PIPELINED: the PE
stream is S(p+1) ... PV(p), so the next pair's S runs during exp(p) and the
ACT engine (the steady-state rate limiter) never waits on the serial
exp->mask->PV->S chain.  K^T is replicated to partitions 64:127 via an
SBUF->SBUF DMA with one t-block of slack.
Epilogue per slot (right after its last PV): PE-transpose O_aug^T ->
[128tq,65] fp32, reciprocal of col 64, scale -> natural [128,64] fp32 rows,
DMA out.  Host reassembles slots.
"""

import numpy as np
import ml_dtypes

BF16 = ml_dtypes.bfloat16

B, T, NE, HD = 4, 4096, 1024, 64
QB = 512            # q-block width
KB = 128            # k-block width
NQB = T // QB       # 8 t-blocks
NT = NE // 128      # 8 n-tiles (projection contraction)
SLOT_NK = [8, 16, 24, 32]          # k-blocks per slot (pairs: 4, 8, 12, 16)
HALF_QBS = [[0, 3, 4, 7], [1, 2, 5, 6]]   # slot si hosts q-block HALF_QBS[h][si]

# pair emission schedule: _SCHED[tb] = [(si, p), ...] emitted after proj(tb)
_SCHED = {tb: [] for tb in range(1, NQB)}
for _tb in range(1, 3):
    _SCHED[_tb] += [(0, p) for p in range(2 * (_tb - 1), 2 * _tb)]
for _tb in range(3, 5):
    _SCHED[_tb] += [(1, p) for p in range(4 * (_tb - 3), 4 * (_tb - 2))]
for _tb in range(5, 7):
    _SCHED[_tb] += [(2, p) for p in range(6 * (_tb - 5), 6 * (_tb - 4))]
for _tb in range(1, 8):
    _SCHED[_tb] += [(3, p) for p in range(2 * (_tb - 1), 2 * _tb)]
_POST = [(3, 14), (3, 15)]

_CACHE = {}


def _build_program():
    import concourse.bass as bass
    import concourse.mybir as mybir
    import concourse.tile as tile

    f32 = mybir.dt.float32
    bf16 = mybir.dt.bfloat16
    AF = mybir.ActivationFunctionType
    MS = bass.MemorySpace
    nc = bass.Bass("TRN2", target_bir_lowering=True, debug=False,
                   enable_asserts=False)

    # all pre-swizzled to [128 partitions, contiguous free]
    xt_d = nc.dram_tensor("xt", [128, NQB * NT * QB], bf16,
                          kind="ExternalInput").ap()
    xq_d = nc.dram_tensor("xq", [128, 4 * NT * QB], bf16,
                          kind="ExternalInput").ap()
    wkv_d = nc.dram_tensor("wkv", [128, NT * 128], bf16,
                           kind="ExternalInput").ap()
    wq_d = nc.dram_tensor("wq", [128, NT * HD], bf16,
                          kind="ExternalInput").ap()
    ident_d = nc.dram_tensor("ident", [128, 128], f32, kind="ExternalInput").ap()
    identh_d = nc.dram_tensor("identh", [128, 64], bf16, kind="ExternalInput").ap()
    masks_d = nc.dram_tensor("masks", [128, 16 * 2 * QB], bf16,
                             kind="ExternalInput").ap()
    out_d = nc.dram_tensor("out", [4 * QB, HD], f32, kind="ExternalOutput").ap()

    with tile.TileContext(nc) as tc:
        with (
            tc.tile_pool(name="consts", bufs=1) as cpool,
            tc.tile_pool(name="big", bufs=1) as bigpool,
            tc.tile_pool(name="xt", bufs=3) as xtpool,
            tc.tile_pool(name="xq", bufs=4) as xqpool,
            tc.tile_pool(name="pt", bufs=4) as ptpool,
            tc.tile_pool(name="osb", bufs=4) as osbpool,
            tc.tile_pool(name="onat", bufs=6) as onatpool,
            tc.tile_pool(name="rec", bufs=6) as recpool,
            tc.tile_pool(name="sps", bufs=2, space=MS.PSUM) as spool,
            tc.tile_pool(name="o3ps", bufs=1, space=MS.PSUM) as o3pool,
            tc.tile_pool(name="ops", bufs=1, space=MS.PSUM) as opool,
            tc.tile_pool(name="projps", bufs=1, space=MS.PSUM) as projpool,
            tc.tile_pool(name="vtps", bufs=1, space=MS.PSUM) as vtpool,
        ):
            # ---- constants (sync queue; gpsimd queue is for the big loads) ----
            wkv_sb = cpool.tile([128, NT, 128], bf16)
            nc.sync.dma_start(wkv_sb[:], wkv_d[:])
            wq_sb = cpool.tile([128, NT, HD], bf16)
            nc.sync.dma_start(wq_sb[:], wq_d[:])
            ident = cpool.tile([128, 128], f32)
            nc.sync.dma_start(ident[:], ident_d[:])
            identh = cpool.tile([128, 64], bf16)
            nc.sync.dma_start(identh[:], identh_d[:])
            # mask tiles are DMA'd per slot (sync queue) near first use so
            # the 4MB doesn't compete with xt/xq in the startup window
            masks = cpool.tile([128, 16 * 2 * QB], bf16)
            mask_loaded = [False] * 4

            def load_masks(si, eng=None):
                if not mask_loaded[si]:
                    mask_loaded[si] = True
                    lo, hi = si * 4 * 2 * QB, (si + 1) * 4 * 2 * QB
                    (eng or nc.sync).dma_start(masks[:, lo:hi],
                                               masks_d[:, lo:hi])

            # ---- persistent sbuf state ----
            kvt = bigpool.tile([128, T], bf16)         # 0:64 K^T, 64:128 V^T
            ktr = bigpool.tile([128, T], bf16)         # 64:128 = K^T replica
            qt_sel = bigpool.tile([128, 4 * QB], bf16) # own Q^T, both halves
            v_aug = bigpool.tile([128, 32 * 65], bf16) # V natural + ones col
            nc.vector.memset(v_aug[:], 1.0)

            o_ps_of = {}

            def emit_epilogue(si):
                o_ps = o_ps_of[si]
                ot_sb = osbpool.tile([65, QB], f32, tag="osb",
                                     name=f"ot{si}")
                nc.vector.tensor_copy(ot_sb[:], o_ps[:])
                for u in range(QB // 128):
                    tp_ps = vtpool.tile([128, QB], f32, tag="vt",
                                        name=f"tp{si}_{u}")
                    nc.tensor.transpose(
                        tp_ps[:, 0:65], ot_sb[:, u * 128:(u + 1) * 128],
                        ident[0:65, 0:65])
                    rec = recpool.tile([128, 1], f32, tag="rec",
                                       name=f"rec{si}_{u}")
                    nc.vector.reciprocal(rec[:], tp_ps[:, 64:65])
                    o_nat = onatpool.tile([128, HD], f32, tag="onat",
                                          name=f"onat{si}_{u}")
                    nc.vector.tensor_scalar(
                        o_nat[:], tp_ps[:, 0:HD], rec[:], None,
                        mybir.AluOpType.mult)
                    nc.sync.dma_start(
                        out_d[si * QB + u * 128: si * QB + (u + 1) * 128, :],
                        o_nat[:])

            def emit_S(si, p):
                npair = SLOT_NK[si] // 2
                ka, kb2 = 2 * p, 2 * p + 1
                s_ps = spool.tile([128, 2 * QB], f32, tag="sps",
                                  name=f"s{si}_{p}")
                nc.tensor.matmul(
                    s_ps[:, 0:QB],
                    kvt[0:64, ka * KB:(ka + 1) * KB],
                    qt_sel[0:64, si * QB:(si + 1) * QB],
                    start=True, stop=True)
                nc.tensor.matmul(
                    s_ps[:, QB:2 * QB],
                    ktr[64:128, kb2 * KB:(kb2 + 1) * KB],
                    qt_sel[64:128, si * QB:(si + 1) * QB],
                    start=True, stop=True)
                pt = ptpool.tile([128, 2 * QB], bf16, tag="pt",
                                 name=f"pt{si}_{p}")
                nc.scalar.activation(pt[:], s_ps[:], AF.Exp,
                                     scale=float(HD) ** -0.5)
                m = p - (npair - 4)
                if m >= 0:
                    load_masks(si)
                    idx = (si * 4 + m) * 2 * QB
                    nc.vector.tensor_tensor(
                        pt[:], pt[:], masks[:, idx:idx + 2 * QB],
                        mybir.AluOpType.mult)
                return pt

            def emit_PV(si, p, pt):
                npair = SLOT_NK[si] // 2
                if p == 0:
                    pool = o3pool if si == 3 else opool
                    o_ps_of[si] = pool.tile(
                        [65, QB], f32, name=f"o_ps{si}",
                        tag="o3" if si == 3 else "ops")
                o_ps = o_ps_of[si]
                ka, kb2 = 2 * p, 2 * p + 1
                nc.tensor.matmul(
                    o_ps[:], v_aug[:, ka * 65:ka * 65 + 65], pt[:, 0:QB],
                    start=(p == 0), stop=False, skip_group_check=True)
                nc.tensor.matmul(
                    o_ps[:], v_aug[:, kb2 * 65:kb2 * 65 + 65],
                    pt[:, QB:2 * QB],
                    start=False, stop=(p == npair - 1),
                    skip_group_check=True)
                if p == npair - 1:
                    epi_pending.append(si)

            # software pipeline: PE stream is S(p+1) ... [epilogue] ... PV(p)
            # so the next pair's S (which gates the next exp) runs during
            # exp(p), and a completed slot's epilogue transposes never delay
            # the S stream -- only the slack-rich PV
            pend = [None]
            epi_pending = []

            def push_pair(si, p):
                pt = emit_S(si, p)
                while epi_pending:
                    emit_epilogue(epi_pending.pop(0))
                if pend[0] is not None:
                    emit_PV(*pend[0])
                pend[0] = (si, p, pt)

            load_masks(0)

            # ---- interleaved input prologue on the gpsimd queue: earliest-
            # needed tensors first, no pool-gated trigger blocks the queue ----
            xq_tiles = []
            xt_tiles = {}

            def xt_load(tb, nchunks):
                xt_sb = xtpool.tile([128, NT, QB], bf16, tag="xt",
                                    name=f"xt{tb}")
                base = tb * NT * QB
                step = NT // nchunks
                for c in range(0, NT, step):
                    nc.gpsimd.dma_start(
                        xt_sb[:, c:c + step, :],
                        xt_d[:, base + c * QB: base + (c + step) * QB])
                xt_tiles[tb] = xt_sb

            def xq_load(si, eng):
                xq_sb = xqpool.tile([128, NT, QB], bf16, tag="xq",
                                    name=f"xq{si}")
                eng.dma_start(xq_sb[:],
                              xq_d[:, si * NT * QB:(si + 1) * NT * QB])
                xq_tiles.append(xq_sb)

            # three independent trigger queues (gpsimd SWDGE + sync/scalar
            # HWDGE) so the early transfers run concurrently on the shared
            # SDMA engines instead of serializing on one ring
            xq_load(0, nc.sync)
            xq_load(1, nc.scalar)
            xt_load(0, 2)
            xq_load(2, nc.sync)
            xq_load(3, nc.scalar)
            xt_load(1, 2)
            xt2_sb = xtpool.tile([128, NT, QB], bf16, tag="xt", name="xt2")
            nc.scalar.dma_start(xt2_sb[:],
                                xt_d[:, 2 * NT * QB:3 * NT * QB])
            xt_tiles[2] = xt2_sb

            # ---- up-front Q projections (only need wq + xq) ----
            for si in range(4):
                xq_sb = xq_tiles[si]
                # col-tiled halves write DIAGONAL slices of a 2-bank tile so
                # each accumulation group owns its psum bank outright
                q2_ps = spool.tile([128, 2 * QB], f32, tag="sps",
                                   name=f"q2_{si}")
                for ni in range(NT):
                    nc.tensor.matmul(q2_ps[0:64, 0:QB], wq_sb[:, ni, :],
                                     xq_sb[:, ni, :],
                                     start=(ni == 0), stop=(ni == NT - 1))
                    nc.tensor.matmul(q2_ps[64:128, QB:2 * QB], wq_sb[:, ni, :],
                                     xq_sb[:, ni, :],
                                     start=(ni == 0), stop=(ni == NT - 1))
                nc.vector.tensor_copy(qt_sel[0:64, si * QB:(si + 1) * QB],
                                      q2_ps[0:64, 0:QB])
                nc.vector.tensor_copy(qt_sel[64:128, si * QB:(si + 1) * QB],
                                      q2_ps[64:128, QB:2 * QB])

            # ---- main pipeline over t-blocks ----
            for tb in range(NQB):
                if tb in xt_tiles:
                    xt_sb = xt_tiles[tb]
                else:
                    xt_load(tb, 2)
                    xt_sb = xt_tiles[tb]
                if tb in (1, 2, 3):
                    load_masks(tb, nc.gpsimd)
                sched = _SCHED.get(tb, [])
                for si, p in sched[:2]:
                    push_pair(si, p)
                kv_ps = projpool.tile([128, QB], f32, tag="proj")
                for ni in range(NT):
                    nc.tensor.matmul(kv_ps[:], wkv_sb[:, ni, :], xt_sb[:, ni, :],
                                     start=(ni == 0), stop=(ni == NT - 1))
                nc.vector.tensor_copy(kvt[:, tb * QB:(tb + 1) * QB], kv_ps[:])
                # replicate K^T to partitions 64:127 for the row-tiled S
                nc.sync.dma_start(ktr[64:128, tb * QB:(tb + 1) * QB],
                                  kvt[0:64, tb * QB:(tb + 1) * QB])
                for j in range(QB // KB):
                    kb = tb * (QB // KB) + j
                    tp_ps = vtpool.tile([128, QB], bf16, tag="vt")
                    nc.tensor.transpose(
                        tp_ps[:, 0:64], kvt[64:128, kb * KB:(kb + 1) * KB],
                        identh[64:128, 0:64])
                    nc.vector.tensor_copy(v_aug[:, kb * 65:kb * 65 + 64],
                                          tp_ps[:, 0:64])
                for si, p in sched[2:]:
                    push_pair(si, p)
            for si, p in _POST:
                push_pair(si, p)
            emit_PV(*pend[0])
            while epi_pending:
                emit_epilogue(epi_pending.pop(0))

    _legalize_matmul_waits(nc)
    return nc


def _legalize_matmul_waits(nc):
    """walrus' LW template encodes at most one sync-wait; hoist extra waits
    from Matmult instructions onto a preceding PE NoOp (same queue, so
    ordering semantics are identical)."""
    import concourse.mybir as mybir

    for f in nc.m.functions:
        for bb in f.blocks:
            new_insts = []
            for inst in bb.instructions:
                si = inst.sync_info
                if (si is not None and si.on_wait and len(si.on_wait) >= 2):
                    for w in si.on_wait:
                        nop = mybir.InstNoOp(
                            name=nc.get_next_instruction_name(),
                            text_hint="wait_hoist", bass_nofuse=True)
                        nop.engine = inst.engine
                        nop.sync_info = mybir.SyncInfo(
                            on_wait=[w], on_update=[])
                        new_insts.append(nop)
                    inst.sync_info = mybir.SyncInfo(
                        on_wait=[], on_update=list(si.on_update or []))
                new_insts.append(inst)
            del bb.instructions[:]
            for i in new_insts:
                bb.instructions.append(i)


def _host_masks(half):
    """16 mask tiles [128, 1024] bf16: slot si, masked pair m covers k-blocks
    kx = nk-8+2m (cols 0:512) and kx+1 (cols 512:1024).
    valid(i, c) iff qoff + c >= kx*128 + i."""
    i = np.arange(128, dtype=np.int32)[:, None]
    c = np.arange(QB, dtype=np.int32)[None, :]
    tiles = []
    for si, nk in enumerate(SLOT_NK):
        qoff = HALF_QBS[half][si] * QB
        for m in range(4):
            for kx in (nk - 8 + 2 * m, nk - 8 + 2 * m + 1):
                tiles.append((qoff + c - kx * 128 - i) >= 0)
    return np.ascontiguousarray(
        np.concatenate(tiles, axis=1).astype(BF16))


def _swizzle(arr):
    """[NE, W] -> [128, (W//QB) * NT * QB] partition-major sbuf layout:
    out[p, (w*NT + nt)*QB + t] = arr[nt*128 + p, w*QB + t]."""
    ne, width = arr.shape
    nw = width // QB
    a = arr.reshape(NT, 128, nw, QB)          # [nt, p, w, t]
    a = a.transpose(1, 2, 0, 3)               # [p, w, nt, t]
    return np.ascontiguousarray(a.reshape(128, nw * NT * QB))


def _swizzle_w(w):
    """[NE, M] -> [128, NT*M]: out[p, nt*M + m] = w[nt*128 + p, m]."""
    m = w.shape[1]
    a = w.reshape(NT, 128, m).transpose(1, 0, 2)
    return np.ascontiguousarray(a.reshape(128, NT * m))


def _make_inputs(x, Wq, Wk, Wv):
    wkv = _swizzle_w(np.concatenate([Wk, Wv], axis=1).astype(BF16))
    wq = _swizzle_w(np.asarray(Wq, dtype=np.float32).astype(BF16))
    ident = np.eye(128, dtype=np.float32)
    identh = np.zeros((128, 64), dtype=np.float32)
    identh[64:128, :] = np.eye(64, dtype=np.float32)
    identh = identh.astype(BF16)

    in_maps = []
    for c in range(8):
        b, half = c // 2, c % 2
        xb = np.asarray(x[b], dtype=np.float32)
        xt = _swizzle(np.ascontiguousarray(xb.T).astype(BF16))
        xq_cols = np.concatenate(
            [xb[qb * QB:(qb + 1) * QB, :].T for qb in HALF_QBS[half]],
            axis=1)
        xq = _swizzle(np.ascontiguousarray(xq_cols).astype(BF16))
        in_maps.append({
            "xt": xt, "xq": xq, "wkv": wkv, "wq": wq, "ident": ident,
            "identh": identh, "masks": _host_masks(half),
        })
    return in_maps


def kernel(x, Wq, Wk, Wv, _want_results=False, _trace=False):
    from concourse import bass_utils

    if "prog" not in _CACHE:
        _CACHE["prog"] = _build_program()
    nc = _CACHE["prog"]
    in_maps = _make_inputs(x, Wq, Wk, Wv)
    res = bass_utils.run_bass_kernel_spmd(nc, in_maps, core_ids=list(range(8)),
                                          trace=_trace)
    out = np.zeros((B, T, HD), dtype=np.float32)
    for c in range(8):
        b, half = c // 2, c % 2
        o = res.results[c]["out"]
        for si in range(4):
            qb = HALF_QBS[half][si]
            out[b, qb * QB:(qb + 1) * QB, :] = o[si * QB:(si + 1) * QB, :]
    if _want_results:
        return out, res
    return out
